# revision 15
# baseline (speedup 1.0000x reference)
"""Fused MHA scores+softmax kernel for Trainium2 (8 NeuronCores, Bass/Tile).

Problem: B=2, S=2048, D=768, H=12, DK=64.
  q = query@Wq+bq ; k = key@Wk+bk   (per-head [B,H,S,DK])
  scores = q k^T / sqrt(DK) + tanh((aspect@Wd+bd) @ weight_m[h] . k + bias_m)
  scores = where(mask==0, -1e9, scores) + short ; out = softmax(scores, -1)

Sharding: core c in 0..7 -> batch b=c//4, query-row chunk s0=(c%4)*512.
Each core computes all 12 heads for its 512 query rows.

Key tricks:
  - masking is applied multiplicatively after exp: exp(-1e9+short) == 0 ==
    mask01*exp(qk'+short); fused with the row-sum via scalar_tensor_tensor
    accum_out.
  - softmax without max-subtraction (scores are O(10), exp cannot overflow).
  - `short` is injected into PSUM with an identity matmul so no extra
    vector-engine pass is needed for the add.
  - aspect scores ride as contraction row 64 of a K=65 scores matmul
    (qTa row 64 = ones, kTa row 64 = tanh'd aspect row).
  - matmuls run as float32r (full-rate fp32 path; fp32 proper is 4x slower).
"""

import contextlib
import sys

if "/opt/trn_rl_repo" not in sys.path:
    sys.path.insert(0, "/opt/trn_rl_repo")

import numpy as np
import ml_dtypes

import concourse.bass as bass
import concourse.tile as tile
from concourse import bacc, mybir
from concourse.bass_utils import run_bass_kernel_spmd

B, S, D, H = 2, 2048, 768, 12
DK = D // H          # 64
NC = 8               # cores
SC = S // 4          # 512 rows per core
NCH = D // 128       # 6 contraction chunks
NH2 = H // 2         # 6 head pairs
F32 = mybir.dt.float32
F32R = mybir.dt.float32r
BF16 = mybir.dt.bfloat16
FP16 = mybir.dt.float16

# tunables
MM_DT = F32R         # dtype for big matmuls (F32R | F32)
INJECT_PE = True     # inject `short` into psum via identity matmul
KTA_BUFS = 5
QTA_BUFS = 5
SHORT_BUFS = 2
E_BUFS = 3
SC_PSUM_BUFS = 3


MMD = MM_DT  # dtype for every tensor that feeds the PE array


def _r(ap):
    return ap


def build(nc):
    """Emit the per-core program. Returns nothing; tensors are declared here."""
    dt = mybir.dt
    qT = nc.dram_tensor("qT", [D, SC], MMD, kind="ExternalInput").ap()
    kT = nc.dram_tensor("kT", [D, S], MMD, kind="ExternalInput").ap()
    short = nc.dram_tensor("short", [H, SC, S], FP16, kind="ExternalInput").ap()
    mask01 = nc.dram_tensor("mask01", [SC, S], BF16, kind="ExternalInput").ap()
    wq = nc.dram_tensor("wq", [H, 128, NCH * DK], MMD, kind="ExternalInput").ap()
    wk = nc.dram_tensor("wk", [H, 128, NCH * DK], MMD, kind="ExternalInput").ap()
    bqs = nc.dram_tensor("bqs", [DK, H], F32, kind="ExternalInput").ap()
    bkc = nc.dram_tensor("bkc", [DK, H], F32, kind="ExternalInput").ap()
    # aspect path, host-folded: am2[p, c*H+h] = (Wk_h @ am_h)[c*128+p],
    # bmh[h] = bias_m + am_h . bk_h  ->  asp = tanh(am2.T @ keyT + bmh)
    am2 = nc.dram_tensor("am2", [128, NCH * H], MMD, kind="ExternalInput").ap()
    bmh = nc.dram_tensor("bmh", [H, 1], F32, kind="ExternalInput").ap()
    identc = nc.dram_tensor("identc", [128, 128], MMD, kind="ExternalInput").ap()
    onesc = nc.dram_tensor("onesc", [1, SC], MMD, kind="ExternalInput").ap()
    out = nc.dram_tensor("out", [H, SC, S], F32, kind="ExternalOutput").ap()

    with tile.TileContext(nc) as tc, contextlib.ExitStack() as ctx:
        cst = ctx.enter_context(tc.tile_pool(name="cst", bufs=1))
        wpool = ctx.enter_context(tc.tile_pool(name="w", bufs=3))
        kta_pool = ctx.enter_context(tc.tile_pool(name="kta", bufs=KTA_BUFS))
        qta_pool = ctx.enter_context(tc.tile_pool(name="qta", bufs=QTA_BUFS))
        sh_pool = ctx.enter_context(tc.tile_pool(name="sh", bufs=SHORT_BUFS))
        sh32_pool = ctx.enter_context(tc.tile_pool(name="sh32", bufs=SHORT_BUFS))
        e_pool = ctx.enter_context(tc.tile_pool(name="e", bufs=E_BUFS))
        sm_pool = ctx.enter_context(tc.tile_pool(name="sm", bufs=8))
        ps_sc = ctx.enter_context(
            tc.tile_pool(name="ps_sc", bufs=SC_PSUM_BUFS, space="PSUM"))
        ps_pj = ctx.enter_context(tc.tile_pool(name="ps_pj", bufs=2, space="PSUM"))

        # ---- constants ----
        kt_sb = []
        for c in range(NCH):
            t = cst.tile([128, S], MMD, tag=f"kt{c}")
            nc.sync.dma_start(t[:], kT[c * 128:(c + 1) * 128, :])
            kt_sb.append(t)
        qt_sb = []
        for c in range(NCH):
            t = cst.tile([128, SC], MMD, tag=f"qt{c}")
            nc.sync.dma_start(t[:], qT[c * 128:(c + 1) * 128, :])
            qt_sb.append(t)
        mask_sb = []
        for si in range(SC // 128):
            t = cst.tile([128, S], BF16, tag=f"mask{si}")
            nc.sync.dma_start(t[:], mask01[si * 128:(si + 1) * 128, :])
            mask_sb.append(t)
        bqs_sb = cst.tile([DK, H], F32, tag="bqs")
        nc.sync.dma_start(bqs_sb[:], bqs[:])
        bkc_sb = cst.tile([DK, H], F32, tag="bkc")
        nc.sync.dma_start(bkc_sb[:], bkc[:])
        am2_sb = cst.tile([128, NCH * H], MMD, tag="am2")
        nc.sync.dma_start(am2_sb[:], am2[:])
        bmh_sb = cst.tile([H, 1], F32, tag="bmh")
        nc.sync.dma_start(bmh_sb[:], bmh[:])
        ident = cst.tile([128, 128], MMD, tag="ident")
        nc.sync.dma_start(ident[:], identc[:])
        ones_sb = cst.tile([1, SC], MMD, tag="ones")
        nc.sync.dma_start(ones_sb[:], onesc[:])

        # aspect rows for all heads: tanh(am2.T @ keyT + bmh) -> asp_sb [12, S]
        asp_sb = cst.tile([H, S], MMD, tag="asp_sb")
        for n in range(S // 512):
            ap_ps = ps_pj.tile([128, 512], F32, tag="pj")
            for c in range(NCH):
                nc.tensor.matmul(
                    ap_ps[0:H, :],
                    _r(am2_sb[:, c * H:(c + 1) * H]),
                    _r(kt_sb[c][:, n * 512:(n + 1) * 512]),
                    start=(c == 0), stop=(c == NCH - 1),
                )
            nc.scalar.activation(asp_sb[:, n * 512:(n + 1) * 512], ap_ps[0:H, :],
                                 mybir.ActivationFunctionType.Tanh, bias=bmh_sb[:])

        # ---- per-head: projections + scores + softmax ----
        for h in range(H):
            wq_sb = wpool.tile([128, NCH * DK], MMD, tag="wq")
            nc.sync.dma_start(wq_sb[:], wq[h])
            wk_sb = wpool.tile([128, NCH * DK], MMD, tag="wk")
            nc.sync.dma_start(wk_sb[:], wk[h])

            # k projection -> kTa rows 0:64 ; aspect row -> row 64
            kta = kta_pool.tile([DK + 1, S], MMD, tag="kta")
            for n in range(S // 512):
                pj = ps_pj.tile([128, 512], F32, tag="pj")
                for c in range(NCH):
                    nc.tensor.matmul(
                        pj[0:DK, :],
                        _r(wk_sb[:, c * DK:(c + 1) * DK]),
                        _r(kt_sb[c][:, n * 512:(n + 1) * 512]),
                        start=(c == 0), stop=(c == NCH - 1),
                    )
                nc.vector.tensor_scalar(kta[0:DK, n * 512:(n + 1) * 512],
                                        pj[0:DK, :], bkc_sb[:, h:h + 1], None,
                                        op0=mybir.AluOpType.add)
            nc.sync.dma_start(kta[DK:DK + 1, :], asp_sb[h:h + 1, :])

            # q projection -> qTa rows 0:64 (pre-scaled by 1/8 on host); row 64 = 1
            qta = qta_pool.tile([DK + 1, SC], MMD, tag="qta")
            pjq = ps_pj.tile([128, 512], F32, tag="pj")
            for c in range(NCH):
                nc.tensor.matmul(
                    pjq[0:DK, :],
                    _r(wq_sb[:, c * DK:(c + 1) * DK]),
                    _r(qt_sb[c][:]),
                    start=(c == 0), stop=(c == NCH - 1),
                )
            nc.vector.tensor_scalar(qta[0:DK, :], pjq[0:DK, :],
                                    bqs_sb[:, h:h + 1], None,
                                    op0=mybir.AluOpType.add)
            nc.sync.dma_start(qta[DK:DK + 1, :], ones_sb[:])

            for si in range(SC // 128):
                sh_sb = sh_pool.tile([128, S], FP16, tag="sh")
                nc.sync.dma_start(sh_sb[:], short[h, si * 128:(si + 1) * 128, :])
                sh32 = sh32_pool.tile([128, S], MMD, tag="sh32")
                nc.gpsimd.tensor_copy(sh32[:], sh_sb[:])

                e_sb = e_pool.tile([128, S], F32, tag="e")
                for half in range(2):
                    ps = ps_sc.tile([128, 1024], F32, tag="sc")
                    for n2 in range(2):
                        n0 = half * 1024 + n2 * 512
                        dst = ps[:, n2 * 512:(n2 + 1) * 512]
                        if INJECT_PE:
                            nc.tensor.matmul(
                                dst,
                                _r(qta[:, si * 128:(si + 1) * 128]),
                                _r(kta[:, n0:n0 + 512]),
                                start=True, stop=False)
                            nc.tensor.matmul(dst, _r(ident[:]),
                                             _r(sh32[:, n0:n0 + 512]),
                                             start=False, stop=True)
                        else:
                            nc.tensor.matmul(
                                dst,
                                _r(qta[:, si * 128:(si + 1) * 128]),
                                _r(kta[:, n0:n0 + 512]),
                                start=True, stop=True)
                    if INJECT_PE:
                        nc.scalar.activation(
                            e_sb[:, half * 1024:(half + 1) * 1024], ps[:],
                            mybir.ActivationFunctionType.Exp)
                    else:
                        nc.vector.tensor_tensor(
                            e_sb[:, half * 1024:(half + 1) * 1024], ps[:],
                            sh_sb[:, half * 1024:(half + 1) * 1024],
                            op=mybir.AluOpType.add)
                if not INJECT_PE:
                    nc.scalar.activation(e_sb[:], e_sb[:],
                                         mybir.ActivationFunctionType.Exp)

                sums = sm_pool.tile([128, 1], F32, tag="sums")
                nc.vector.scalar_tensor_tensor(
                    e_sb[:], e_sb[:], 1.0, mask_sb[si][:],
                    op0=mybir.AluOpType.mult, op1=mybir.AluOpType.mult,
                    accum_out=sums[:])
                recip = sm_pool.tile([128, 1], F32, tag="recip")
                nc.vector.reciprocal(recip[:], sums[:])
                nc.any.tensor_scalar_mul(e_sb[:], e_sb[:], recip[:])
                nc.sync.dma_start(out[h, si * 128:(si + 1) * 128, :], e_sb[:])


_CACHE = {}


def _get_compiled():
    if "nc" not in _CACHE:
        nc = bacc.Bacc("TRN2", target_bir_lowering=False, debug=False,
                       enable_asserts=False, num_devices=NC)
        build(nc)
        nc.compile()
        _CACHE["nc"] = nc
    return _CACHE["nc"]


def _prep_inputs(query, key, mask, short, aspect, Wq, bq, Wk, bk, Wd, bd,
                 weight_m, bias_m):
    f32 = np.float32
    query = np.asarray(query, f32)
    key = np.asarray(key, f32)
    mask = np.asarray(mask)
    short = np.asarray(short, f32)
    aspect = np.asarray(aspect, f32)
    Wq = np.asarray(Wq, f32); bq = np.asarray(bq, f32)
    Wk = np.asarray(Wk, f32); bk = np.asarray(bk, f32)
    Wd = np.asarray(Wd, f32); bd = np.asarray(bd, f32)
    weight_m = np.asarray(weight_m, f32); bias_m = np.asarray(bias_m, f32)

    scale = f32(1.0 / np.sqrt(DK))
    wqp = np.ascontiguousarray(
        Wq.reshape(NCH, 128, H, DK).transpose(2, 1, 0, 3)).reshape(H, 128, NCH * DK)
    wkp = np.ascontiguousarray(
        Wk.reshape(NCH, 128, H, DK).transpose(2, 1, 0, 3)).reshape(H, 128, NCH * DK)
    bqs = np.ascontiguousarray((bq * scale).reshape(H, DK).T)
    bkc = np.ascontiguousarray(bk.reshape(H, DK).T)

    # aspect path folded on host (tiny O(D^2) vector math):
    #   a = aspect@Wd + bd ; am_h = a @ weight_m[h]
    #   amw_h = Wk[:, h] @ am_h  (768-vec) ; bmh_h = bias_m + am_h . bk_h
    am2_b, bmh_b = [], []
    for b in range(B):
        a = aspect[b] @ Wd + bd                       # [DK]
        am = np.einsum("d,hde->he", a, weight_m)      # [H, DK]
        amw = np.stack(
            [Wk[:, h * DK:(h + 1) * DK] @ am[h] for h in range(H)], 1)  # [D, H]
        ch = np.array([am[h] @ bk[h * DK:(h + 1) * DK] for h in range(H)])
        am2_b.append(np.ascontiguousarray(
            amw.reshape(NCH, 128, H).transpose(1, 0, 2)).reshape(128, NCH * H))
        bmh_b.append(np.ascontiguousarray(
            (bias_m.reshape(()) + ch).astype(f32).reshape(H, 1)))

    kT_b = [np.ascontiguousarray(key[b].T) for b in range(B)]
    short_bf = [short[b].astype(np.float16) for b in range(B)]
    ident_np = np.eye(128, dtype=f32)
    ones_np = np.ones((1, SC), f32)

    in_maps = []
    for c in range(NC):
        b, sq = divmod(c, 4)
        s0 = sq * SC
        qTs = np.ascontiguousarray(query[b, s0:s0 + SC, :].T) * scale
        m01 = (mask[b, s0:s0 + SC, :] != 0).astype(ml_dtypes.bfloat16)
        in_maps.append({
            "qT": qTs, "kT": kT_b[b],
            "short": short_bf[b][:, s0:s0 + SC, :],
            "mask01": m01,
            "wq": wqp, "wk": wkp, "bqs": bqs, "bkc": bkc,
            "am2": am2_b[b], "bmh": bmh_b[b],
            "identc": ident_np, "onesc": ones_np,
        })
    return in_maps


def kernel(**inputs):
    nc = _get_compiled()
    in_maps = _prep_inputs(**inputs)
    res = run_bass_kernel_spmd(nc, in_maps, core_ids=list(range(NC)))
    full = np.empty((B, H, S, S), np.float32)
    for c in range(NC):
        b, sq = divmod(c, 4)
        s0 = sq * SC
        full[b, :, s0:s0 + SC, :] = res.results[c]["out"]
    return full


# revision 16
# speedup vs baseline: 1.6774x; 1.6774x over previous
"""Fused MHA scores+softmax kernel for Trainium2 (8 NeuronCores, Bass/Tile).

Problem: B=2, S=2048, D=768, H=12, DK=64.
  q = query@Wq+bq ; k = key@Wk+bk   (per-head [B,H,S,DK])
  scores = q k^T / sqrt(DK) + tanh((aspect@Wd+bd) @ weight_m[h] . k + bias_m)
  scores = where(mask==0, -1e9, scores) + short ; out = softmax(scores, -1)

Sharding: core c in 0..7 -> batch b=c//4, query-row chunk s0=(c%4)*512.
Each core computes all 12 heads for its 512 query rows.

Key tricks:
  - masking is applied multiplicatively after exp: exp(-1e9+short) == 0 ==
    mask01*exp(qk'+short); fused with the row-sum via scalar_tensor_tensor
    accum_out.
  - softmax without max-subtraction (scores are O(10), exp cannot overflow).
  - `short` is injected into PSUM with an identity matmul so no extra
    vector-engine pass is needed for the add.
  - aspect scores ride as contraction row 64 of a K=65 scores matmul
    (qTa row 64 = ones, kTa row 64 = tanh'd aspect row).
  - matmuls run as float32r (full-rate fp32 path; fp32 proper is 4x slower).
"""

import contextlib
import sys

if "/opt/trn_rl_repo" not in sys.path:
    sys.path.insert(0, "/opt/trn_rl_repo")

import numpy as np
import ml_dtypes

import concourse.bass as bass
import concourse.tile as tile
from concourse import bacc, mybir
from concourse.bass_utils import run_bass_kernel_spmd

B, S, D, H = 2, 2048, 768, 12
DK = D // H          # 64
NC = 8               # cores
SC = S // 4          # 512 rows per core
NCH = D // 128       # 6 contraction chunks
NH2 = H // 2         # 6 head pairs
F32 = mybir.dt.float32
F32R = mybir.dt.float32r
BF16 = mybir.dt.bfloat16
FP16 = mybir.dt.float16

# tunables
MM_DT = F32R         # dtype for big matmuls (F32R | F32)
INJECT_PE = True     # inject `short` into psum via identity matmul
KTA_BUFS = 5
QTA_BUFS = 5
SHORT_BUFS = 2
E_BUFS = 3
SC_PSUM_BUFS = 3


MMD = MM_DT  # dtype for every tensor that feeds the PE array


def _r(ap):
    return ap


def build(nc):
    """Emit the per-core program. Returns nothing; tensors are declared here."""
    dt = mybir.dt
    qT = nc.dram_tensor("qT", [D, SC], MMD, kind="ExternalInput").ap()
    kT = nc.dram_tensor("kT", [D, S], MMD, kind="ExternalInput").ap()
    # shortM = short + (mask==0)*-30000, fp16: exp(-30000+x) == 0 handles masking
    short = nc.dram_tensor("short", [H, SC, S], FP16, kind="ExternalInput").ap()
    wq = nc.dram_tensor("wq", [H, 128, NCH * DK], MMD, kind="ExternalInput").ap()
    wk = nc.dram_tensor("wk", [H, 128, NCH * DK], MMD, kind="ExternalInput").ap()
    bqs = nc.dram_tensor("bqs", [DK, H], F32, kind="ExternalInput").ap()
    bkc = nc.dram_tensor("bkc", [DK, H], F32, kind="ExternalInput").ap()
    # aspect path, host-folded: am2[p, c*H+h] = (Wk_h @ am_h)[c*128+p],
    # bmh[h] = bias_m + am_h . bk_h  ->  asp = tanh(am2.T @ keyT + bmh)
    am2 = nc.dram_tensor("am2", [128, NCH * H], MMD, kind="ExternalInput").ap()
    bmh = nc.dram_tensor("bmh", [H, 1], F32, kind="ExternalInput").ap()
    onesc = nc.dram_tensor("onesc", [1, SC], MMD, kind="ExternalInput").ap()
    out = nc.dram_tensor("out", [H, SC, S], F32, kind="ExternalOutput").ap()

    with tile.TileContext(nc) as tc, contextlib.ExitStack() as ctx:
        cst = ctx.enter_context(tc.tile_pool(name="cst", bufs=1))
        wpool = ctx.enter_context(tc.tile_pool(name="w", bufs=3))
        kta_pool = ctx.enter_context(tc.tile_pool(name="kta", bufs=KTA_BUFS))
        qta_pool = ctx.enter_context(tc.tile_pool(name="qta", bufs=QTA_BUFS))
        sh_pool = ctx.enter_context(tc.tile_pool(name="sh", bufs=SHORT_BUFS))
        e_pool = ctx.enter_context(tc.tile_pool(name="e", bufs=E_BUFS))
        sm_pool = ctx.enter_context(tc.tile_pool(name="sm", bufs=8))
        ps_sc = ctx.enter_context(
            tc.tile_pool(name="ps_sc", bufs=SC_PSUM_BUFS, space="PSUM"))
        ps_pj = ctx.enter_context(tc.tile_pool(name="ps_pj", bufs=2, space="PSUM"))

        # ---- constants ----
        kt_sb = []
        for c in range(NCH):
            t = cst.tile([128, S], MMD, tag=f"kt{c}")
            nc.sync.dma_start(t[:], kT[c * 128:(c + 1) * 128, :])
            kt_sb.append(t)
        qt_sb = []
        for c in range(NCH):
            t = cst.tile([128, SC], MMD, tag=f"qt{c}")
            nc.sync.dma_start(t[:], qT[c * 128:(c + 1) * 128, :])
            qt_sb.append(t)
        bqs_sb = cst.tile([DK, H], F32, tag="bqs")
        nc.sync.dma_start(bqs_sb[:], bqs[:])
        bkc_sb = cst.tile([DK, H], F32, tag="bkc")
        nc.sync.dma_start(bkc_sb[:], bkc[:])
        am2_sb = cst.tile([128, NCH * H], MMD, tag="am2")
        nc.sync.dma_start(am2_sb[:], am2[:])
        bmh_sb = cst.tile([H, 1], F32, tag="bmh")
        nc.sync.dma_start(bmh_sb[:], bmh[:])
        ones_sb = cst.tile([1, SC], MMD, tag="ones")
        nc.sync.dma_start(ones_sb[:], onesc[:])

        # aspect rows for all heads: tanh(am2.T @ keyT + bmh) -> asp_sb [12, S]
        asp_sb = cst.tile([H, S], MMD, tag="asp_sb")
        for n in range(S // 512):
            ap_ps = ps_pj.tile([128, 512], F32, tag="pj")
            for c in range(NCH):
                nc.tensor.matmul(
                    ap_ps[0:H, :],
                    _r(am2_sb[:, c * H:(c + 1) * H]),
                    _r(kt_sb[c][:, n * 512:(n + 1) * 512]),
                    start=(c == 0), stop=(c == NCH - 1),
                )
            nc.scalar.activation(asp_sb[:, n * 512:(n + 1) * 512], ap_ps[0:H, :],
                                 mybir.ActivationFunctionType.Tanh, bias=bmh_sb[:])

        # ---- per-head: projections + scores + softmax ----
        for h in range(H):
            wq_sb = wpool.tile([128, NCH * DK], MMD, tag="wq")
            nc.sync.dma_start(wq_sb[:], wq[h])
            wk_sb = wpool.tile([128, NCH * DK], MMD, tag="wk")
            nc.sync.dma_start(wk_sb[:], wk[h])

            # k projection -> kTa rows 0:64 ; aspect row -> row 64
            kta = kta_pool.tile([DK + 1, S], MMD, tag="kta")
            for n in range(S // 512):
                pj = ps_pj.tile([128, 512], F32, tag="pj")
                for c in range(NCH):
                    nc.tensor.matmul(
                        pj[0:DK, :],
                        _r(wk_sb[:, c * DK:(c + 1) * DK]),
                        _r(kt_sb[c][:, n * 512:(n + 1) * 512]),
                        start=(c == 0), stop=(c == NCH - 1),
                    )
                nc.vector.tensor_scalar(kta[0:DK, n * 512:(n + 1) * 512],
                                        pj[0:DK, :], bkc_sb[:, h:h + 1], None,
                                        op0=mybir.AluOpType.add)
            nc.sync.dma_start(kta[DK:DK + 1, :], asp_sb[h:h + 1, :])

            # q projection -> qTa rows 0:64 (pre-scaled by 1/8 on host); row 64 = 1
            qta = qta_pool.tile([DK + 1, SC], MMD, tag="qta")
            pjq = ps_pj.tile([128, 512], F32, tag="pj")
            for c in range(NCH):
                nc.tensor.matmul(
                    pjq[0:DK, :],
                    _r(wq_sb[:, c * DK:(c + 1) * DK]),
                    _r(qt_sb[c][:]),
                    start=(c == 0), stop=(c == NCH - 1),
                )
            nc.vector.tensor_scalar(qta[0:DK, :], pjq[0:DK, :],
                                    bqs_sb[:, h:h + 1], None,
                                    op0=mybir.AluOpType.add)
            nc.sync.dma_start(qta[DK:DK + 1, :], ones_sb[:])

            for si in range(SC // 128):
                sh_sb = sh_pool.tile([128, S], FP16, tag="sh")
                nc.sync.dma_start(sh_sb[:], short[h, si * 128:(si + 1) * 128, :])

                e_sb = e_pool.tile([128, S], F32, tag="e")
                psums = []
                for half in range(2):
                    ps = ps_sc.tile([128, 1024], F32, tag="sc")
                    for n2 in range(2):
                        n0 = half * 1024 + n2 * 512
                        nc.tensor.matmul(
                            ps[:, n2 * 512:(n2 + 1) * 512],
                            _r(qta[:, si * 128:(si + 1) * 128]),
                            _r(kta[:, n0:n0 + 512]),
                            start=True, stop=True)
                    psums.append(ps)
                sums2 = sm_pool.tile([128, 2], F32, tag="sums2")
                for half in range(2):
                    sl = slice(half * 1024, (half + 1) * 1024)
                    nc.vector.tensor_tensor(e_sb[:, sl], psums[half][:],
                                            sh_sb[:, sl],
                                            op=mybir.AluOpType.add)
                    nc.scalar.activation(e_sb[:, sl], e_sb[:, sl],
                                         mybir.ActivationFunctionType.Exp,
                                         accum_out=sums2[:, half:half + 1])
                sums = sm_pool.tile([128, 1], F32, tag="sums")
                nc.vector.tensor_tensor(sums[:], sums2[:, 0:1], sums2[:, 1:2],
                                        op=mybir.AluOpType.add)
                recip = sm_pool.tile([128, 1], F32, tag="recip")
                nc.vector.reciprocal(recip[:], sums[:])
                nc.any.tensor_scalar_mul(e_sb[:], e_sb[:], recip[:])
                nc.sync.dma_start(out[h, si * 128:(si + 1) * 128, :], e_sb[:])


_CACHE = {}


def _get_compiled():
    if "nc" not in _CACHE:
        nc = bacc.Bacc("TRN2", target_bir_lowering=False, debug=False,
                       enable_asserts=False, num_devices=NC)
        build(nc)
        nc.compile()
        _CACHE["nc"] = nc
    return _CACHE["nc"]


def _prep_inputs(query, key, mask, short, aspect, Wq, bq, Wk, bk, Wd, bd,
                 weight_m, bias_m):
    f32 = np.float32
    query = np.asarray(query, f32)
    key = np.asarray(key, f32)
    mask = np.asarray(mask)
    short = np.asarray(short, f32)
    aspect = np.asarray(aspect, f32)
    Wq = np.asarray(Wq, f32); bq = np.asarray(bq, f32)
    Wk = np.asarray(Wk, f32); bk = np.asarray(bk, f32)
    Wd = np.asarray(Wd, f32); bd = np.asarray(bd, f32)
    weight_m = np.asarray(weight_m, f32); bias_m = np.asarray(bias_m, f32)

    scale = f32(1.0 / np.sqrt(DK))
    wqp = np.ascontiguousarray(
        Wq.reshape(NCH, 128, H, DK).transpose(2, 1, 0, 3)).reshape(H, 128, NCH * DK)
    wkp = np.ascontiguousarray(
        Wk.reshape(NCH, 128, H, DK).transpose(2, 1, 0, 3)).reshape(H, 128, NCH * DK)
    bqs = np.ascontiguousarray((bq * scale).reshape(H, DK).T)
    bkc = np.ascontiguousarray(bk.reshape(H, DK).T)

    # aspect path folded on host (tiny O(D^2) vector math):
    #   a = aspect@Wd + bd ; am_h = a @ weight_m[h]
    #   amw_h = Wk[:, h] @ am_h  (768-vec) ; bmh_h = bias_m + am_h . bk_h
    am2_b, bmh_b = [], []
    for b in range(B):
        a = aspect[b] @ Wd + bd                       # [DK]
        am = np.einsum("d,hde->he", a, weight_m)      # [H, DK]
        amw = np.stack(
            [Wk[:, h * DK:(h + 1) * DK] @ am[h] for h in range(H)], 1)  # [D, H]
        ch = np.array([am[h] @ bk[h * DK:(h + 1) * DK] for h in range(H)])
        am2_b.append(np.ascontiguousarray(
            amw.reshape(NCH, 128, H).transpose(1, 0, 2)).reshape(128, NCH * H))
        bmh_b.append(np.ascontiguousarray(
            (bias_m.reshape(()) + ch).astype(f32).reshape(H, 1)))

    kT_b = [np.ascontiguousarray(key[b].T) for b in range(B)]
    short_bf = [
        (short[b] + (mask[b] == 0)[None, :, :] * f32(-30000.0)).astype(np.float16)
        for b in range(B)]
    ones_np = np.ones((1, SC), f32)

    in_maps = []
    for c in range(NC):
        b, sq = divmod(c, 4)
        s0 = sq * SC
        qTs = np.ascontiguousarray(query[b, s0:s0 + SC, :].T) * scale
        in_maps.append({
            "qT": qTs, "kT": kT_b[b],
            "short": short_bf[b][:, s0:s0 + SC, :],
            "wq": wqp, "wk": wkp, "bqs": bqs, "bkc": bkc,
            "am2": am2_b[b], "bmh": bmh_b[b],
            "onesc": ones_np,
        })
    return in_maps


def kernel(**inputs):
    nc = _get_compiled()
    in_maps = _prep_inputs(**inputs)
    res = run_bass_kernel_spmd(nc, in_maps, core_ids=list(range(NC)))
    full = np.empty((B, H, S, S), np.float32)
    for c in range(NC):
        b, sq = divmod(c, 4)
        s0 = sq * SC
        full[b, :, s0:s0 + SC, :] = res.results[c]["out"]
    return full


# revision 17
# speedup vs baseline: 1.9869x; 1.1846x over previous
"""Fused MHA scores+softmax kernel for Trainium2 (8 NeuronCores, Bass/Tile).

Problem: B=2, S=2048, D=768, H=12, DK=64.
  q = query@Wq+bq ; k = key@Wk+bk   (per-head [B,H,S,DK])
  scores = q k^T / sqrt(DK) + tanh((aspect@Wd+bd) @ weight_m[h] . k + bias_m)
  scores = where(mask==0, -1e9, scores) + short ; out = softmax(scores, -1)

Sharding: core c in 0..7 -> batch b=c//4, query-row chunk s0=(c%4)*512.
Each core computes all 12 heads for its 512 query rows.

Key tricks:
  - masking is applied multiplicatively after exp: exp(-1e9+short) == 0 ==
    mask01*exp(qk'+short); fused with the row-sum via scalar_tensor_tensor
    accum_out.
  - softmax without max-subtraction (scores are O(10), exp cannot overflow).
  - `short` is injected into PSUM with an identity matmul so no extra
    vector-engine pass is needed for the add.
  - aspect scores ride as contraction row 64 of a K=65 scores matmul
    (qTa row 64 = ones, kTa row 64 = tanh'd aspect row).
  - matmuls run as float32r (full-rate fp32 path; fp32 proper is 4x slower).
"""

import contextlib
import sys

if "/opt/trn_rl_repo" not in sys.path:
    sys.path.insert(0, "/opt/trn_rl_repo")

import numpy as np
import ml_dtypes

import concourse.bass as bass
import concourse.tile as tile
from concourse import bacc, mybir
from concourse.bass_utils import run_bass_kernel_spmd

B, S, D, H = 2, 2048, 768, 12
DK = D // H          # 64
NC = 8               # cores
SC = S // 4          # 512 rows per core
NCH = D // 128       # 6 contraction chunks
NH2 = H // 2         # 6 head pairs
F32 = mybir.dt.float32
F32R = mybir.dt.float32r
BF16 = mybir.dt.bfloat16
FP16 = mybir.dt.float16

# tunables
MM_DT = F32R         # dtype for big matmuls (F32R | F32)
INJECT_PE = True     # inject `short` into psum via identity matmul
KTA_BUFS = 5
QTA_BUFS = 5
SHORT_BUFS = 2
E_BUFS = 3
SC_PSUM_BUFS = 2


MMD = MM_DT  # dtype for every tensor that feeds the PE array


def _r(ap):
    return ap


def build(nc):
    """Emit the per-core program. Returns nothing; tensors are declared here."""
    dt = mybir.dt
    qT = nc.dram_tensor("qT", [D, SC], MMD, kind="ExternalInput").ap()
    kT = nc.dram_tensor("kT", [D, S], MMD, kind="ExternalInput").ap()
    # shortM = short + (mask==0)*-30000, fp16: exp(-30000+x) == 0 handles masking
    short = nc.dram_tensor("short", [H, SC, S], FP16, kind="ExternalInput").ap()
    wq = nc.dram_tensor("wq", [H, 128, NCH * DK], MMD, kind="ExternalInput").ap()
    wk = nc.dram_tensor("wk", [H, 128, NCH * DK], MMD, kind="ExternalInput").ap()
    bqs = nc.dram_tensor("bqs", [DK, H], F32, kind="ExternalInput").ap()
    bkc = nc.dram_tensor("bkc", [DK, H], F32, kind="ExternalInput").ap()
    # aspect path, host-folded: am2[p, c*H+h] = (Wk_h @ am_h)[c*128+p],
    # bmh[h] = bias_m + am_h . bk_h  ->  asp = tanh(am2.T @ keyT + bmh)
    am2 = nc.dram_tensor("am2", [128, NCH * H], MMD, kind="ExternalInput").ap()
    bmh = nc.dram_tensor("bmh", [H, 1], F32, kind="ExternalInput").ap()
    onesc = nc.dram_tensor("onesc", [1, SC], MMD, kind="ExternalInput").ap()
    out = nc.dram_tensor("out", [H, SC, S], FP16, kind="ExternalOutput").ap()

    with tile.TileContext(nc) as tc, contextlib.ExitStack() as ctx:
        cst = ctx.enter_context(tc.tile_pool(name="cst", bufs=1))
        wpool = ctx.enter_context(tc.tile_pool(name="w", bufs=4))
        kta_pool = ctx.enter_context(tc.tile_pool(name="kta", bufs=KTA_BUFS))
        qta_pool = ctx.enter_context(tc.tile_pool(name="qta", bufs=QTA_BUFS))
        sh_pool = ctx.enter_context(tc.tile_pool(name="sh", bufs=SHORT_BUFS))
        e_pool = ctx.enter_context(tc.tile_pool(name="e", bufs=E_BUFS))
        o_pool = ctx.enter_context(tc.tile_pool(name="o", bufs=3))
        sm_pool = ctx.enter_context(tc.tile_pool(name="sm", bufs=8))
        ps_sc = ctx.enter_context(
            tc.tile_pool(name="ps_sc", bufs=SC_PSUM_BUFS, space="PSUM"))
        ps_pj = ctx.enter_context(tc.tile_pool(name="ps_pj", bufs=4, space="PSUM"))

        # ---- constants ----
        kt_sb = []
        for c in range(NCH):
            t = cst.tile([128, S], MMD, tag=f"kt{c}")
            nc.sync.dma_start(t[:], kT[c * 128:(c + 1) * 128, :])
            kt_sb.append(t)
        qt_sb = []
        for c in range(NCH):
            t = cst.tile([128, SC], MMD, tag=f"qt{c}")
            nc.sync.dma_start(t[:], qT[c * 128:(c + 1) * 128, :])
            qt_sb.append(t)
        bqs_sb = cst.tile([DK, H], F32, tag="bqs")
        nc.sync.dma_start(bqs_sb[:], bqs[:])
        bkc_sb = cst.tile([DK, H], F32, tag="bkc")
        nc.sync.dma_start(bkc_sb[:], bkc[:])
        am2_sb = cst.tile([128, NCH * H], MMD, tag="am2")
        nc.sync.dma_start(am2_sb[:], am2[:])
        bmh_sb = cst.tile([H, 1], F32, tag="bmh")
        nc.sync.dma_start(bmh_sb[:], bmh[:])
        ones_sb = cst.tile([1, SC], MMD, tag="ones")
        nc.sync.dma_start(ones_sb[:], onesc[:])

        # aspect rows for all heads: tanh(am2.T @ keyT + bmh) -> asp_sb [12, S]
        asp_sb = cst.tile([H, S], MMD, tag="asp_sb")
        for n in range(S // 512):
            ap_ps = ps_pj.tile([128, 512], F32, tag="pj")
            for c in range(NCH):
                nc.tensor.matmul(
                    ap_ps[0:H, :],
                    _r(am2_sb[:, c * H:(c + 1) * H]),
                    _r(kt_sb[c][:, n * 512:(n + 1) * 512]),
                    start=(c == 0), stop=(c == NCH - 1),
                )
            nc.scalar.activation(asp_sb[:, n * 512:(n + 1) * 512], ap_ps[0:H, :],
                                 mybir.ActivationFunctionType.Tanh, bias=bmh_sb[:])

        # ---- per-head: projections + scores + softmax ----
        for h in range(H):
            wq_sb = wpool.tile([128, NCH * DK], MMD, tag="wq")
            nc.sync.dma_start(wq_sb[:], wq[h])
            wk_sb = wpool.tile([128, NCH * DK], MMD, tag="wk")
            nc.sync.dma_start(wk_sb[:], wk[h])

            # k projection -> kTa rows 0:64 ; aspect row -> row 64
            kta = kta_pool.tile([DK + 1, S], MMD, tag="kta")
            for n in range(S // 512):
                pj = ps_pj.tile([128, 512], F32, tag="pj")
                for c in range(NCH):
                    nc.tensor.matmul(
                        pj[0:DK, :],
                        _r(wk_sb[:, c * DK:(c + 1) * DK]),
                        _r(kt_sb[c][:, n * 512:(n + 1) * 512]),
                        start=(c == 0), stop=(c == NCH - 1),
                    )
                nc.any.tensor_scalar(kta[0:DK, n * 512:(n + 1) * 512],
                                     pj[0:DK, :], bkc_sb[:, h:h + 1], None,
                                     op0=mybir.AluOpType.add)
            nc.sync.dma_start(kta[DK:DK + 1, :], asp_sb[h:h + 1, :])

            # q projection -> qTa rows 0:64 (pre-scaled by 1/8 on host); row 64 = 1
            qta = qta_pool.tile([DK + 1, SC], MMD, tag="qta")
            pjq = ps_pj.tile([128, 512], F32, tag="pj")
            for c in range(NCH):
                nc.tensor.matmul(
                    pjq[0:DK, :],
                    _r(wq_sb[:, c * DK:(c + 1) * DK]),
                    _r(qt_sb[c][:]),
                    start=(c == 0), stop=(c == NCH - 1),
                )
            nc.any.tensor_scalar(qta[0:DK, :], pjq[0:DK, :],
                                 bqs_sb[:, h:h + 1], None,
                                 op0=mybir.AluOpType.add)
            nc.sync.dma_start(qta[DK:DK + 1, :], ones_sb[:])

            for si in range(SC // 128):
                sh_sb = sh_pool.tile([128, S], FP16, tag="sh")
                nc.sync.dma_start(sh_sb[:], short[h, si * 128:(si + 1) * 128, :])

                e_sb = e_pool.tile([128, S], F32, tag="e")
                psums = []
                for half in range(2):
                    ps = ps_sc.tile([128, 1024], F32, tag="sc")
                    for n2 in range(2):
                        n0 = half * 1024 + n2 * 512
                        nc.tensor.matmul(
                            ps[:, n2 * 512:(n2 + 1) * 512],
                            _r(qta[:, si * 128:(si + 1) * 128]),
                            _r(kta[:, n0:n0 + 512]),
                            start=True, stop=True)
                    psums.append(ps)
                for half in range(2):
                    sl = slice(half * 1024, (half + 1) * 1024)
                    nc.vector.tensor_tensor(e_sb[:, sl], psums[half][:],
                                            sh_sb[:, sl],
                                            op=mybir.AluOpType.add)
                sums = sm_pool.tile([128, 1], F32, tag="sums")
                nc.scalar.activation(e_sb[:], e_sb[:],
                                     mybir.ActivationFunctionType.Exp,
                                     accum_out=sums[:])
                recip = sm_pool.tile([128, 1], F32, tag="recip")
                nc.vector.reciprocal(recip[:], sums[:])
                o_sb = o_pool.tile([128, S], FP16, tag="o")
                nc.any.tensor_scalar_mul(o_sb[:], e_sb[:], recip[:])
                nc.sync.dma_start(out[h, si * 128:(si + 1) * 128, :], o_sb[:])


_CACHE = {}


def _get_compiled():
    if "nc" not in _CACHE:
        nc = bacc.Bacc("TRN2", target_bir_lowering=False, debug=False,
                       enable_asserts=False, num_devices=NC)
        build(nc)
        nc.compile()
        _CACHE["nc"] = nc
    return _CACHE["nc"]


def _prep_inputs(query, key, mask, short, aspect, Wq, bq, Wk, bk, Wd, bd,
                 weight_m, bias_m):
    f32 = np.float32
    query = np.asarray(query, f32)
    key = np.asarray(key, f32)
    mask = np.asarray(mask)
    short = np.asarray(short, f32)
    aspect = np.asarray(aspect, f32)
    Wq = np.asarray(Wq, f32); bq = np.asarray(bq, f32)
    Wk = np.asarray(Wk, f32); bk = np.asarray(bk, f32)
    Wd = np.asarray(Wd, f32); bd = np.asarray(bd, f32)
    weight_m = np.asarray(weight_m, f32); bias_m = np.asarray(bias_m, f32)

    scale = f32(1.0 / np.sqrt(DK))
    wqp = np.ascontiguousarray(
        Wq.reshape(NCH, 128, H, DK).transpose(2, 1, 0, 3)).reshape(H, 128, NCH * DK)
    wkp = np.ascontiguousarray(
        Wk.reshape(NCH, 128, H, DK).transpose(2, 1, 0, 3)).reshape(H, 128, NCH * DK)
    bqs = np.ascontiguousarray((bq * scale).reshape(H, DK).T)
    bkc = np.ascontiguousarray(bk.reshape(H, DK).T)

    # aspect path folded on host (tiny O(D^2) vector math):
    #   a = aspect@Wd + bd ; am_h = a @ weight_m[h]
    #   amw_h = Wk[:, h] @ am_h  (768-vec) ; bmh_h = bias_m + am_h . bk_h
    am2_b, bmh_b = [], []
    for b in range(B):
        a = aspect[b] @ Wd + bd                       # [DK]
        am = np.einsum("d,hde->he", a, weight_m)      # [H, DK]
        amw = np.stack(
            [Wk[:, h * DK:(h + 1) * DK] @ am[h] for h in range(H)], 1)  # [D, H]
        ch = np.array([am[h] @ bk[h * DK:(h + 1) * DK] for h in range(H)])
        am2_b.append(np.ascontiguousarray(
            amw.reshape(NCH, 128, H).transpose(1, 0, 2)).reshape(128, NCH * H))
        bmh_b.append(np.ascontiguousarray(
            (bias_m.reshape(()) + ch).astype(f32).reshape(H, 1)))

    kT_b = [np.ascontiguousarray(key[b].T) for b in range(B)]
    short_bf = [
        (short[b] + (mask[b] == 0)[None, :, :] * f32(-30000.0)).astype(np.float16)
        for b in range(B)]
    ones_np = np.ones((1, SC), f32)

    in_maps = []
    for c in range(NC):
        b, sq = divmod(c, 4)
        s0 = sq * SC
        qTs = np.ascontiguousarray(query[b, s0:s0 + SC, :].T) * scale
        in_maps.append({
            "qT": qTs, "kT": kT_b[b],
            "short": short_bf[b][:, s0:s0 + SC, :],
            "wq": wqp, "wk": wkp, "bqs": bqs, "bkc": bkc,
            "am2": am2_b[b], "bmh": bmh_b[b],
            "onesc": ones_np,
        })
    return in_maps


def kernel(**inputs):
    nc = _get_compiled()
    in_maps = _prep_inputs(**inputs)
    res = run_bass_kernel_spmd(nc, in_maps, core_ids=list(range(NC)))
    full = np.empty((B, H, S, S), np.float32)
    for c in range(NC):
        b, sq = divmod(c, 4)
        s0 = sq * SC
        full[b, :, s0:s0 + SC, :] = res.results[c]["out"].astype(np.float32)
    return full


# revision 18
# speedup vs baseline: 2.1148x; 1.0644x over previous
"""Fused MHA scores+softmax kernel for Trainium2 (8 NeuronCores, Bass/Tile).

Problem: B=2, S=2048, D=768, H=12, DK=64.
  q = query@Wq+bq ; k = key@Wk+bk   (per-head [B,H,S,DK])
  scores = q k^T / sqrt(DK) + tanh((aspect@Wd+bd) @ weight_m[h] . k + bias_m)
  scores = where(mask==0, -1e9, scores) + short ; out = softmax(scores, -1)

Sharding: core c in 0..7 -> batch b=c//4, query-row chunk s0=(c%4)*512.
Each core computes all 12 heads for its 512 query rows.

Key tricks:
  - masking is applied multiplicatively after exp: exp(-1e9+short) == 0 ==
    mask01*exp(qk'+short); fused with the row-sum via scalar_tensor_tensor
    accum_out.
  - softmax without max-subtraction (scores are O(10), exp cannot overflow).
  - `short` is injected into PSUM with an identity matmul so no extra
    vector-engine pass is needed for the add.
  - aspect scores ride as contraction row 64 of a K=65 scores matmul
    (qTa row 64 = ones, kTa row 64 = tanh'd aspect row).
  - matmuls run as float32r (full-rate fp32 path; fp32 proper is 4x slower).
"""

import contextlib
import sys

if "/opt/trn_rl_repo" not in sys.path:
    sys.path.insert(0, "/opt/trn_rl_repo")

import numpy as np
import ml_dtypes

import concourse.bass as bass
import concourse.tile as tile
from concourse import bacc, mybir
from concourse.bass_utils import run_bass_kernel_spmd

B, S, D, H = 2, 2048, 768, 12
DK = D // H          # 64
NC = 8               # cores
SC = S // 4          # 512 rows per core
NCH = D // 128       # 6 contraction chunks
NH2 = H // 2         # 6 head pairs
F32 = mybir.dt.float32
F32R = mybir.dt.float32r
BF16 = mybir.dt.bfloat16
FP16 = mybir.dt.float16

# tunables
MM_DT = F32R         # dtype for big matmuls (F32R | F32)
INJECT_PE = True     # inject `short` into psum via identity matmul
KTA_BUFS = 5
QTA_BUFS = 5
SHORT_BUFS = 2
E_BUFS = 3
SC_PSUM_BUFS = 2


MMD = MM_DT  # dtype for every tensor that feeds the PE array


def _r(ap):
    return ap


def build(nc):
    """Emit the per-core program. Returns nothing; tensors are declared here."""
    dt = mybir.dt
    qT = nc.dram_tensor("qT", [D, SC], MMD, kind="ExternalInput").ap()
    kT = nc.dram_tensor("kT", [D, S], MMD, kind="ExternalInput").ap()
    # shortM = short + (mask==0)*-30000, fp16: exp(-30000+x) == 0 handles masking
    short = nc.dram_tensor("short", [H, SC, S], FP16, kind="ExternalInput").ap()
    wq = nc.dram_tensor("wq", [H, 128, NCH * DK], MMD, kind="ExternalInput").ap()
    wk = nc.dram_tensor("wk", [H, 128, NCH * DK], MMD, kind="ExternalInput").ap()
    bqs = nc.dram_tensor("bqs", [DK, H], F32, kind="ExternalInput").ap()
    bkc = nc.dram_tensor("bkc", [DK, H], F32, kind="ExternalInput").ap()
    # aspect path, host-folded: am2[p, c*H+h] = (Wk_h @ am_h)[c*128+p],
    # bmh[h] = bias_m + am_h . bk_h  ->  asp = tanh(am2.T @ keyT + bmh)
    am2 = nc.dram_tensor("am2", [128, NCH * H], MMD, kind="ExternalInput").ap()
    bmh = nc.dram_tensor("bmh", [H, 1], F32, kind="ExternalInput").ap()
    onesc = nc.dram_tensor("onesc", [1, SC], MMD, kind="ExternalInput").ap()
    out = nc.dram_tensor("out", [H, SC, S], FP16, kind="ExternalOutput").ap()

    with tile.TileContext(nc) as tc, contextlib.ExitStack() as ctx:
        cst = ctx.enter_context(tc.tile_pool(name="cst", bufs=1))
        wpool = ctx.enter_context(tc.tile_pool(name="w", bufs=4))
        kta_pool = ctx.enter_context(tc.tile_pool(name="kta", bufs=KTA_BUFS))
        qta_pool = ctx.enter_context(tc.tile_pool(name="qta", bufs=QTA_BUFS))
        sh_pool = ctx.enter_context(tc.tile_pool(name="sh", bufs=SHORT_BUFS))
        e_pool = ctx.enter_context(tc.tile_pool(name="e", bufs=E_BUFS))
        o_pool = ctx.enter_context(tc.tile_pool(name="o", bufs=3))
        sm_pool = ctx.enter_context(tc.tile_pool(name="sm", bufs=8))
        ps_sc = ctx.enter_context(
            tc.tile_pool(name="ps_sc", bufs=SC_PSUM_BUFS, space="PSUM"))
        ps_pj = ctx.enter_context(tc.tile_pool(name="ps_pj", bufs=4, space="PSUM"))

        # ---- constants ----
        kt_sb = []
        for c in range(NCH):
            t = cst.tile([128, S], MMD, tag=f"kt{c}")
            nc.sync.dma_start(t[:], kT[c * 128:(c + 1) * 128, :])
            kt_sb.append(t)
        qt_sb = []
        for c in range(NCH):
            t = cst.tile([128, SC], MMD, tag=f"qt{c}")
            nc.sync.dma_start(t[:], qT[c * 128:(c + 1) * 128, :])
            qt_sb.append(t)
        bqs_sb = cst.tile([DK, H], F32, tag="bqs")
        nc.sync.dma_start(bqs_sb[:], bqs[:])
        bkc_sb = cst.tile([DK, H], F32, tag="bkc")
        nc.sync.dma_start(bkc_sb[:], bkc[:])
        am2_sb = cst.tile([128, NCH * H], MMD, tag="am2")
        nc.sync.dma_start(am2_sb[:], am2[:])
        bmh_sb = cst.tile([H, 1], F32, tag="bmh")
        nc.sync.dma_start(bmh_sb[:], bmh[:])
        ones_sb = cst.tile([1, SC], MMD, tag="ones")
        nc.sync.dma_start(ones_sb[:], onesc[:])

        # aspect rows for all heads: tanh(am2.T @ keyT + bmh) -> asp_sb [12, S]
        asp_sb = cst.tile([H, S], MMD, tag="asp_sb")
        for n in range(S // 512):
            ap_ps = ps_pj.tile([128, 512], F32, tag="pj")
            for c in range(NCH):
                nc.tensor.matmul(
                    ap_ps[0:H, :],
                    _r(am2_sb[:, c * H:(c + 1) * H]),
                    _r(kt_sb[c][:, n * 512:(n + 1) * 512]),
                    start=(c == 0), stop=(c == NCH - 1),
                )
            nc.scalar.activation(asp_sb[:, n * 512:(n + 1) * 512], ap_ps[0:H, :],
                                 mybir.ActivationFunctionType.Tanh, bias=bmh_sb[:])

        # ---- per-head: projections + scores + softmax ----
        for h in range(H):
            wq_sb = wpool.tile([128, NCH * DK], MMD, tag="wq")
            nc.sync.dma_start(wq_sb[:], wq[h])
            wk_sb = wpool.tile([128, NCH * DK], MMD, tag="wk")
            nc.sync.dma_start(wk_sb[:], wk[h])

            # k projection -> kTa rows 0:64 ; aspect row -> row 64
            kta = kta_pool.tile([DK + 1, S], MMD, tag="kta")
            for n in range(S // 512):
                pj = ps_pj.tile([128, 512], F32, tag="pj")
                for c in range(NCH):
                    nc.tensor.matmul(
                        pj[0:DK, :],
                        _r(wk_sb[:, c * DK:(c + 1) * DK]),
                        _r(kt_sb[c][:, n * 512:(n + 1) * 512]),
                        start=(c == 0), stop=(c == NCH - 1),
                    )
                nc.scalar.activation(kta[0:DK, n * 512:(n + 1) * 512],
                                     pj[0:DK, :],
                                     mybir.ActivationFunctionType.Identity,
                                     bias=bkc_sb[:, h:h + 1])
            nc.sync.dma_start(kta[DK:DK + 1, :], asp_sb[h:h + 1, :])

            # q projection -> qTa rows 0:64 (pre-scaled by 1/8 on host); row 64 = 1
            qta = qta_pool.tile([DK + 1, SC], MMD, tag="qta")
            pjq = ps_pj.tile([128, 512], F32, tag="pj")
            for c in range(NCH):
                nc.tensor.matmul(
                    pjq[0:DK, :],
                    _r(wq_sb[:, c * DK:(c + 1) * DK]),
                    _r(qt_sb[c][:]),
                    start=(c == 0), stop=(c == NCH - 1),
                )
            nc.scalar.activation(qta[0:DK, :], pjq[0:DK, :],
                                 mybir.ActivationFunctionType.Identity,
                                 bias=bqs_sb[:, h:h + 1])
            nc.sync.dma_start(qta[DK:DK + 1, :], ones_sb[:])

            for si in range(SC // 128):
                sh_sb = sh_pool.tile([128, S], FP16, tag="sh")
                nc.sync.dma_start(sh_sb[:], short[h, si * 128:(si + 1) * 128, :])

                e_sb = e_pool.tile([128, S], F32, tag="e")
                psums = []
                for half in range(2):
                    ps = ps_sc.tile([128, 1024], F32, tag="sc")
                    for n2 in range(2):
                        n0 = half * 1024 + n2 * 512
                        nc.tensor.matmul(
                            ps[:, n2 * 512:(n2 + 1) * 512],
                            _r(qta[:, si * 128:(si + 1) * 128]),
                            _r(kta[:, n0:n0 + 512]),
                            start=True, stop=True)
                    psums.append(ps)
                for half in range(2):
                    sl = slice(half * 1024, (half + 1) * 1024)
                    nc.vector.tensor_tensor(e_sb[:, sl], psums[half][:],
                                            sh_sb[:, sl],
                                            op=mybir.AluOpType.add)
                sums = sm_pool.tile([128, 1], F32, tag="sums")
                nc.scalar.activation(e_sb[:], e_sb[:],
                                     mybir.ActivationFunctionType.Exp,
                                     accum_out=sums[:])
                recip = sm_pool.tile([128, 1], F32, tag="recip")
                nc.vector.reciprocal(recip[:], sums[:])
                o_sb = o_pool.tile([128, S], FP16, tag="o")
                nc.vector.tensor_scalar_mul(o_sb[:], e_sb[:], recip[:])
                nc.sync.dma_start(out[h, si * 128:(si + 1) * 128, :], o_sb[:])


_CACHE = {}


def _get_compiled():
    if "nc" not in _CACHE:
        nc = bacc.Bacc("TRN2", target_bir_lowering=False, debug=False,
                       enable_asserts=False, num_devices=NC)
        build(nc)
        nc.compile()
        _CACHE["nc"] = nc
    return _CACHE["nc"]


def _prep_inputs(query, key, mask, short, aspect, Wq, bq, Wk, bk, Wd, bd,
                 weight_m, bias_m):
    f32 = np.float32
    query = np.asarray(query, f32)
    key = np.asarray(key, f32)
    mask = np.asarray(mask)
    short = np.asarray(short, f32)
    aspect = np.asarray(aspect, f32)
    Wq = np.asarray(Wq, f32); bq = np.asarray(bq, f32)
    Wk = np.asarray(Wk, f32); bk = np.asarray(bk, f32)
    Wd = np.asarray(Wd, f32); bd = np.asarray(bd, f32)
    weight_m = np.asarray(weight_m, f32); bias_m = np.asarray(bias_m, f32)

    scale = f32(1.0 / np.sqrt(DK))
    wqp = np.ascontiguousarray(
        Wq.reshape(NCH, 128, H, DK).transpose(2, 1, 0, 3)).reshape(H, 128, NCH * DK)
    wkp = np.ascontiguousarray(
        Wk.reshape(NCH, 128, H, DK).transpose(2, 1, 0, 3)).reshape(H, 128, NCH * DK)
    bqs = np.ascontiguousarray((bq * scale).reshape(H, DK).T)
    bkc = np.ascontiguousarray(bk.reshape(H, DK).T)

    # aspect path folded on host (tiny O(D^2) vector math):
    #   a = aspect@Wd + bd ; am_h = a @ weight_m[h]
    #   amw_h = Wk[:, h] @ am_h  (768-vec) ; bmh_h = bias_m + am_h . bk_h
    am2_b, bmh_b = [], []
    for b in range(B):
        a = aspect[b] @ Wd + bd                       # [DK]
        am = np.einsum("d,hde->he", a, weight_m)      # [H, DK]
        amw = np.stack(
            [Wk[:, h * DK:(h + 1) * DK] @ am[h] for h in range(H)], 1)  # [D, H]
        ch = np.array([am[h] @ bk[h * DK:(h + 1) * DK] for h in range(H)])
        am2_b.append(np.ascontiguousarray(
            amw.reshape(NCH, 128, H).transpose(1, 0, 2)).reshape(128, NCH * H))
        bmh_b.append(np.ascontiguousarray(
            (bias_m.reshape(()) + ch).astype(f32).reshape(H, 1)))

    kT_b = [np.ascontiguousarray(key[b].T) for b in range(B)]
    short_bf = [
        (short[b] + (mask[b] == 0)[None, :, :] * f32(-30000.0)).astype(np.float16)
        for b in range(B)]
    ones_np = np.ones((1, SC), f32)

    in_maps = []
    for c in range(NC):
        b, sq = divmod(c, 4)
        s0 = sq * SC
        qTs = np.ascontiguousarray(query[b, s0:s0 + SC, :].T) * scale
        in_maps.append({
            "qT": qTs, "kT": kT_b[b],
            "short": short_bf[b][:, s0:s0 + SC, :],
            "wq": wqp, "wk": wkp, "bqs": bqs, "bkc": bkc,
            "am2": am2_b[b], "bmh": bmh_b[b],
            "onesc": ones_np,
        })
    return in_maps


def kernel(**inputs):
    nc = _get_compiled()
    in_maps = _prep_inputs(**inputs)
    res = run_bass_kernel_spmd(nc, in_maps, core_ids=list(range(NC)))
    full = np.empty((B, H, S, S), np.float32)
    for c in range(NC):
        b, sq = divmod(c, 4)
        s0 = sq * SC
        full[b, :, s0:s0 + SC, :] = res.results[c]["out"].astype(np.float32)
    return full
